# revision 27
# baseline (speedup 1.0000x reference)
"""Trainium2 Bass kernel for a 2-layer sparse GAT (nn_GAT_71889162600962).

Strategy (8 NeuronCores, SPMD):
- Nodes striped across cores (12500/core, padded to stripe=12544=98*128).
  Edges sharded by the core that owns their *src* node, so each core
  exclusively owns the segment sums (num/denom) of its stripe.
- Per layer, each core computes its stripe of h2 = h @ W (and the two
  attention projections s_src/s_dst = h2 . a halves) with bf16 PE matmuls,
  transposes h2 back to row-major "records"
      rec[n] = [h2[n] (256 bf16) | s_dst[n] | 1.0 | pad] (768B rows)
  and all-gathers the record table across cores.
- Edge phase: edges grouped by 128-src-node chunk (98 chunks/core, chunk c
  = nodes [128c, 128c+128)), each chunk's edges split into 4 cells by dst
  quadrant of the 100352-row table (so gather indices fit int16), each
  cell padded to a fixed CAP slots with trailing -1 (skipped by ucode).
  One bulk dma_gather per cell fetches all records at full DMA rate.
  e = exp(-leakyrelu(s_src+s_dst)) on DVE/ACT; per 128-slot tile a one-hot
  scatter matrix M[slot,s] = (iota==src_local)*e built in one DVE
  tensor_scalar, and PE matmuls M.T @ X accumulate num/denom in PSUM.
  Finalize (num/denom, ELU) per chunk; rows written out contiguously.
- All per-core variation lives in input index arrays, so one SPMD program
  serves all 8 cores.
"""

import math

import numpy as np
import ml_dtypes

P = 128
D = 256
REC_W = 384              # record row (bf16): 256 h2 | s_dst | 1.0 | pad
NCORES = 8
NQ = 4                   # dst-table quadrants (int16 gather index limit)
NEG_SLOPE = 0.2

SWDGE_QUEUES = 4         # parallel Q7 descriptor-generation queues

_IOTA_F = np.tile(np.arange(P, dtype=np.float32)[None, :], (P, 1))
_IDENT_BF = np.eye(P, dtype=np.float32).astype(ml_dtypes.bfloat16)


def _cfg(n_nodes):
    npc = n_nodes // NCORES
    stripe = math.ceil((npc + 44) / P) * P
    return npc, stripe, stripe * NCORES


# ---------------------------------------------------------------------------
# Host-side preprocessing
# ---------------------------------------------------------------------------

def _prep(edges, n_nodes):
    """Build per-core gather-index / src-col arrays.

    Returns (xidx [8, NCH, 128, NQ, CAP//16] int16 wrap16-replicated,
             scol [8, NCH, 128, TPC] float32 (-1 = pad slot), CAP).
    """
    npc, stripe, vfull = _cfg(n_nodes)
    qrows = vfull // NQ
    nch = stripe // P

    src = np.asarray(edges[0]).astype(np.int64)
    dst = np.asarray(edges[1]).astype(np.int64)
    dst_g = (dst // npc) * stripe + dst % npc
    q = dst_g // qrows
    lidx = (dst_g % qrows).astype(np.int16)
    core = src // npc
    src_l = src - core * npc
    chunk = src_l >> 7
    scol_v = (src_l & 127).astype(np.float32)

    ngroup = NCORES * nch * NQ
    key = (core * nch + chunk) * NQ + q
    order = np.argsort(key, kind="stable")
    ks = key[order]
    counts = np.bincount(ks, minlength=ngroup)
    cap = int(math.ceil(counts.max() / P) * P)
    tcell = cap // P
    tpc = NQ * tcell
    starts = np.zeros(ngroup + 1, np.int64)
    np.cumsum(counts, out=starts[1:])
    pos = np.arange(len(ks), dtype=np.int64) - starts[ks]

    # pad slots get a VALID dummy index (0), never -1: the gather ucode
    # trims trailing negatives and emits fewer descriptors than the
    # decode-side ring reservation (computed from num_idxs_reg), which
    # corrupts the SWDGE ring when the two round to different
    # 128-multiples.  scol stays -1 on pad slots so they contribute 0.
    xidx = np.zeros((ngroup, cap), np.int16)
    xidx[ks, pos] = lidx[order]

    scol = np.full((NCORES * nch * P, tpc), -1.0, np.float32)
    cchunk = ks // NQ
    slot = (ks % NQ) * cap + pos
    scol[cchunk * P + slot % P, slot // P] = scol_v[order]
    scol = scol.reshape(NCORES, nch, P, tpc)

    # wrap16: [G, cap] -> [G, 16, cap//16] -> replicate to 128 partitions
    w = xidx.reshape(ngroup, cap // 16, 16).transpose(0, 2, 1)
    w = np.broadcast_to(w[:, None, :, :], (ngroup, 8, 16, cap // 16))
    w = w.reshape(NCORES, nch, NQ, P, cap // 16).transpose(0, 1, 3, 2, 4)
    return np.ascontiguousarray(w), np.ascontiguousarray(scol), cap


# ---------------------------------------------------------------------------
# Device program
# ---------------------------------------------------------------------------

def _build_program(n_nodes, cap):
    import concourse.bacc as bacc
    import concourse.mybir as mybir
    import concourse.tile as tile

    f32 = mybir.dt.float32
    bf16 = mybir.dt.bfloat16
    i16 = mybir.dt.int16
    Alu = mybir.AluOpType
    Act = mybir.ActivationFunctionType

    npc, stripe, vfull = _cfg(n_nodes)
    qrows = vfull // NQ
    nch = stripe // P
    tcell = cap // P
    tpc = NQ * tcell
    NT = 512
    groups = [list(range(NCORES))]

    nc = bacc.Bacc("TRN2", target_bir_lowering=False, debug=False,
                   num_devices=NCORES, num_swdge_queues=SWDGE_QUEUES)

    embT_d = nc.dram_tensor("embT", [D, stripe], bf16, kind="ExternalInput")
    iota_d = nc.dram_tensor("iotaf", [P, P], f32, kind="ExternalInput")
    ident_d = nc.dram_tensor("identbf", [P, P], bf16, kind="ExternalInput")
    W_d = [nc.dram_tensor(f"W{L + 1}", [D, D], bf16, kind="ExternalInput")
           for L in range(2)]
    Wa_d = [nc.dram_tensor(f"Wa{L + 1}", [D, 2], bf16, kind="ExternalInput")
            for L in range(2)]
    xidx_d = nc.dram_tensor("xidx", [nch, P, NQ, cap // 16], i16,
                            kind="ExternalInput")
    scol_d = nc.dram_tensor("scol", [nch, P, tpc], f32, kind="ExternalInput")
    out_d = nc.dram_tensor("out_stripe", [stripe, D], f32,
                           kind="ExternalOutput")

    rec_stripe = [nc.dram_tensor(f"rec_stripe{L}", [stripe, REC_W], bf16)
                  for L in range(2)]
    rec_full = [nc.dram_tensor(f"rec_full{L}", [vfull, REC_W], bf16,
                               addr_space="Shared") for L in range(2)]
    ssrc_row = [nc.dram_tensor(f"ssrc_row{L}", [1, stripe], bf16)
                for L in range(2)]
    out1rec = nc.dram_tensor("out1rec", [stripe, D], bf16)

    with tile.TileContext(nc) as tc:
        with tc.tile_pool(name="const", bufs=1) as cpool:
            iota_f = cpool.tile([P, P], f32)
            nc.sync.dma_start(iota_f[:], iota_d[:])
            ident = cpool.tile([P, P], bf16)
            nc.sync.dma_start(ident[:], ident_d[:])
            ones1 = cpool.tile([1, P], bf16)
            nc.vector.memset(ones1[:], 1.0)
            W_sb, Wa_sb = [], []
            for L in range(2):
                w = cpool.tile([P, 2, D], bf16)
                wa = cpool.tile([P, 2, 2], bf16)
                for kc in range(2):
                    nc.sync.dma_start(w[:, kc, :], W_d[L][P * kc:P * (kc + 1)])
                    nc.sync.dma_start(wa[:, kc, :],
                                      Wa_d[L][P * kc:P * (kc + 1)])
                W_sb.append(w)
                Wa_sb.append(wa)

            for L in range(2):
                # ---------------- phase A: stripe matmul ------------------
                with (
                    tc.tile_pool(name=f"A{L}", bufs=3) as ap,
                    tc.tile_pool(name=f"As{L}", bufs=1) as spl,
                    tc.tile_pool(name=f"Ap{L}", bufs=2, space="PSUM") as pp,
                    tc.tile_pool(name=f"ApT{L}", bufs=4, space="PSUM") as ppT,
                ):
                    s_sbuf = spl.tile([3, stripe], f32)
                    nc.vector.memset(s_sbuf[:], 1.0)
                    for c0 in range(0, stripe, NT):
                        nsz = min(NT, stripe - c0)
                        hT = []
                        for kc in range(2):
                            t = ap.tile([P, nsz], bf16, tag="hT")
                            if L == 0:
                                nc.sync.dma_start(
                                    t[:], embT_d[P * kc:P * (kc + 1),
                                                 c0:c0 + nsz])
                            else:
                                nc.sync.dma_start_transpose(
                                    t[:], out1rec[c0:c0 + nsz,
                                                  P * kc:P * (kc + 1)])
                            hT.append(t)
                        ps_s = pp.tile([2, NT], f32, space="PSUM", tag="ps_s")
                        for kc in range(2):
                            nc.tensor.matmul(ps_s[:, :nsz],
                                             lhsT=Wa_sb[L][:, kc, :],
                                             rhs=hT[kc][:], start=kc == 0,
                                             stop=kc == 1)
                        nc.vector.tensor_copy(s_sbuf[0:2, c0:c0 + nsz],
                                              ps_s[:, :nsz])
                        rows = [ap.tile([P, D], bf16, tag=f"rows{b}",
                                        name=f"rows{b}")
                                for b in range(nsz // P)]
                        for j in range(2):
                            ps_h = pp.tile([P, NT], f32, space="PSUM",
                                           tag="ps_h")
                            for kc in range(2):
                                nc.tensor.matmul(
                                    ps_h[:, :nsz],
                                    lhsT=W_sb[L][:, kc, P * j:P * (j + 1)],
                                    rhs=hT[kc][:], start=kc == 0, stop=kc == 1)
                            h2T = ap.tile([P, nsz], bf16, tag="h2T")
                            nc.vector.tensor_copy(h2T[:], ps_h[:, :nsz])
                            for b in range(nsz // P):
                                psT = ppT.tile([P, P], bf16, space="PSUM",
                                               tag="psT")
                                nc.tensor.transpose(
                                    out=psT[:], in_=h2T[:, P * b:P * (b + 1)],
                                    identity=ident[:])
                                nc.vector.tensor_copy(
                                    rows[b][:, P * j:P * (j + 1)], psT[:])
                        for b in range(nsz // P):
                            nc.sync.dma_start(
                                rec_stripe[L][c0 + P * b:c0 + P * (b + 1),
                                              :D],
                                rows[b][:])
                    # s_src -> row table; s_dst, ones -> record cols 256/257
                    # (bf16 convert on DVE so the column writes stay on
                    # HWDGE -- mainline gpsimd dma_start would share the
                    # SWDGE ring with the dma_gather ucode)
                    s_bf = spl.tile([3, stripe], bf16)
                    nc.vector.tensor_copy(s_bf[:], s_sbuf[0:3, :])
                    nc.sync.dma_start(ssrc_row[L][0:1, :], s_bf[0:1, :])
                    nc.sync.dma_start(rec_stripe[L][:, D:D + 1], s_bf[1:2, :])
                    nc.sync.dma_start(rec_stripe[L][:, D + 1:D + 2],
                                      s_bf[2:3, :])
                    nc.gpsimd.collective_compute(
                        "AllGather", Alu.bypass, replica_groups=groups,
                        ins=[rec_stripe[L][:]], outs=[rec_full[L][:]])

                # ---------------- phase B: edge phase ---------------------
                tgt = out1rec if L == 0 else out_d
                stage_dt = bf16 if L == 0 else f32
                with (
                    tc.tile_pool(name=f"B{L}", bufs=3) as ep,
                    tc.tile_pool(name=f"Bx{L}", bufs=2) as xp,
                    tc.tile_pool(name=f"Bm{L}", bufs=6) as mp,
                    tc.tile_pool(name=f"Bf{L}", bufs=2) as fp,
                    tc.tile_pool(name=f"Bp{L}", bufs=2, space="PSUM") as pnp,
                    tc.tile_pool(name=f"Bs{L}", bufs=2, space="PSUM") as psb,
                ):
                    # pre-zero the X ring so pad slots always hold finite
                    # bf16 (gather skips trailing -1 indices)
                    for _ in range(2):
                        xt = xp.tile([P, tpc, REC_W], bf16, tag="X")
                        nc.vector.memset(xt[:], 0.0)
                    for ch in range(nch):
                        ix = ep.tile([P, NQ, cap // 16], i16, tag="ix")
                        nc.sync.dma_start(ix[:], xidx_d[ch])
                        scl = ep.tile([P, tpc], f32, tag="scl")
                        nc.sync.dma_start(scl[:], scol_d[ch])
                        # s_rep[p, col] = s_src[128*ch + col] for every p
                        s_row = ep.tile([1, P], bf16, tag="s_row")
                        nc.sync.dma_start(s_row[:],
                                          ssrc_row[L][0:1,
                                                      P * ch:P * (ch + 1)])
                        ps_b = psb.tile([P, P], f32, space="PSUM", tag="ps_b")
                        nc.tensor.matmul(ps_b[:], lhsT=ones1[:],
                                         rhs=s_row[:], start=True, stop=True)
                        s_rep = ep.tile([P, P], f32, tag="s_rep")
                        nc.vector.tensor_copy(s_rep[:], ps_b[:])

                        X = xp.tile([P, tpc, REC_W], bf16, tag="X")
                        if True:
                            for q in range(NQ):
                                # single_packet=False: one packet per desc --
                                # coalescing >64 descs/lane into one packet
                                # (cap>1008) breaks the SDMA engines
                                nc.gpsimd.dma_gather(
                                    X[:, q * tcell:(q + 1) * tcell, :],
                                    rec_full[L][q * qrows:(q + 1) * qrows, :],
                                    ix[:, q, :], cap, cap, REC_W,
                                    single_packet=False,
                                    queue_num=q % SWDGE_QUEUES)

                        # e = exp(-leakyrelu(s_src + s_dst)) for all slots.
                        # s_src per slot via fused one-hot row-reduce:
                        # s_slot[p,t] = sum_col (iota==scol[p,t]) * s_rep
                        ev = ep.tile([P, tpc], f32, tag="ev")
                        if True:
                            s_slot = ep.tile([P, tpc], f32, tag="s_slot")
                            junk = ep.tile([P, P], f32, tag="junk")
                            for t in range(tpc):
                                nc.vector.scalar_tensor_tensor(
                                    out=junk[:], in0=iota_f[:],
                                    scalar=scl[:, t:t + 1], in1=s_rep[:],
                                    op0=Alu.is_equal, op1=Alu.mult,
                                    accum_out=s_slot[:, t:t + 1])
                            sd = ep.tile([P, tpc], f32, tag="sd")
                            nc.vector.tensor_copy(sd[:], X[:, :, D])
                            sc_ = ep.tile([P, tpc], f32, tag="sc_")
                            nc.vector.tensor_tensor(
                                out=sc_[:], in0=sd[:], in1=s_slot[:],
                                op=Alu.add)
                            lr = ep.tile([P, tpc], f32, tag="lr")
                            nc.vector.tensor_scalar(
                                out=lr[:], in0=sc_[:], scalar1=NEG_SLOPE,
                                scalar2=None, op0=Alu.mult)
                            nc.vector.tensor_tensor(
                                out=lr[:], in0=sc_[:], in1=lr[:], op=Alu.max)
                            nc.scalar.activation(ev[:], lr[:], Act.Exp,
                                                 scale=-1.0)

                        psum = pnp.tile([P, D + 2], f32, space="PSUM",
                                        tag="psum")
                        if True:
                            for t in range(tpc):
                                m = mp.tile([P, P], bf16, tag="m")
                                nc.vector.tensor_scalar(
                                    out=m[:], in0=iota_f[:],
                                    scalar1=scl[:, t:t + 1],
                                    scalar2=ev[:, t:t + 1],
                                    op0=Alu.is_equal, op1=Alu.mult)
                                nc.tensor.matmul(
                                    psum[:], lhsT=m[:],
                                    rhs=X[:, t, :D + 2],
                                    start=t == 0, stop=t == tpc - 1)
                        den = fp.tile([P, 1], f32, tag="den")
                        nc.vector.tensor_scalar(
                            out=den[:], in0=psum[:, D + 1:D + 2],
                            scalar1=1e-30, scalar2=None, op0=Alu.max)
                        recip = fp.tile([P, 1], f32, tag="recip")
                        nc.vector.reciprocal(recip[:], den[:])
                        qv = fp.tile([P, D], f32, tag="qv")
                        nc.vector.tensor_scalar_mul(qv[:], psum[:, :D],
                                                    recip[:, :1])
                        amin = fp.tile([P, D], f32, tag="amin")
                        nc.vector.tensor_scalar(
                            out=amin[:], in0=qv[:], scalar1=0.0,
                            scalar2=None, op0=Alu.min)
                        ea = fp.tile([P, D], f32, tag="ea")
                        nc.scalar.activation(ea[:], amin[:], Act.Exp)
                        bmax = fp.tile([P, D], f32, tag="bmax")
                        nc.vector.tensor_scalar(
                            out=bmax[:], in0=qv[:], scalar1=0.0,
                            scalar2=-1.0, op0=Alu.max, op1=Alu.add)
                        stage = fp.tile([P, D], stage_dt, tag="stage")
                        nc.vector.tensor_tensor(
                            out=stage[:], in0=ea[:], in1=bmax[:],
                            op=Alu.add)
                        nc.scalar.dma_start(tgt[P * ch:P * (ch + 1), :],
                                            stage[:])
    nc.compile()
    return nc


# ---------------------------------------------------------------------------
# Persistent-jit PJRT runner (NTFF profiling is unavailable under this axon
# setup, so steady-state re-execution wall clock is the timing source).
# ---------------------------------------------------------------------------

class _Runner:
    def __init__(self, nc, n_cores):
        import jax
        from jax.sharding import Mesh, NamedSharding, PartitionSpec
        from jax.experimental.shard_map import shard_map
        import concourse.mybir as mybir
        from concourse import bass2jax

        bass2jax.install_neuronx_cc_hook()
        self.n_cores = n_cores
        in_names, out_names, out_avals, zero_outs = [], [], [], []
        for alloc in nc.m.functions[0].allocations:
            if not isinstance(alloc, mybir.MemoryLocationSet):
                continue
            name = alloc.memorylocations[0].name
            if alloc.kind == "ExternalInput":
                in_names.append(name)
            elif alloc.kind == "ExternalOutput":
                out_names.append(name)
                shape = tuple(alloc.tensor_shape)
                dtype = mybir.dt.np(alloc.dtype)
                out_avals.append(jax.core.ShapedArray(shape, dtype))
                zero_outs.append(np.zeros(shape, dtype))
        self.partition_name = (nc.partition_id_tensor.name
                               if nc.partition_id_tensor else None)
        if self.partition_name and self.partition_name in in_names:
            in_names.remove(self.partition_name)
        self.in_names = in_names
        self.out_names = out_names
        self.out_avals = out_avals
        self.zero_outs = zero_outs
        n_params = len(in_names)
        self.n_params = n_params
        all_names = in_names + out_names
        if self.partition_name:
            all_names = all_names + [self.partition_name]

        def _body(*args):
            operands = list(args)
            if self.partition_name:
                operands.append(bass2jax.partition_id_tensor())
            return tuple(bass2jax._bass_exec_p.bind(
                *operands, out_avals=tuple(out_avals),
                in_names=tuple(all_names), out_names=tuple(out_names),
                lowering_input_output_aliases=(),
                sim_require_finite=True, sim_require_nnan=True, nc=nc))

        devices = jax.devices()[:n_cores]
        mesh = Mesh(np.asarray(devices), ("core",))
        self.sharding = NamedSharding(mesh, PartitionSpec("core"))
        n_out = len(out_names)
        self.jitted = jax.jit(
            shard_map(_body, mesh=mesh,
                      in_specs=(PartitionSpec("core"),) * (n_params + n_out),
                      out_specs=(PartitionSpec("core"),) * n_out,
                      check_rep=False),
            keep_unused=True)
        self._jax = jax

    def prepare(self, in_maps):
        per_core = [[np.asarray(m[n]) for n in self.in_names]
                    for m in in_maps]
        concat_in = [
            np.concatenate([per_core[c][i] for c in range(self.n_cores)], 0)
            for i in range(self.n_params)]
        concat_zeros = [
            np.zeros((self.n_cores * z.shape[0], *z.shape[1:]), z.dtype)
            for z in self.zero_outs]
        # device-resident args: run() then measures device execution, not
        # host->device transfer of ~100MB of index tables per call
        args = [self._jax.device_put(a, self.sharding)
                for a in concat_in + concat_zeros]
        self._jax.block_until_ready(args)
        return args

    def run(self, args):
        outs = self.jitted(*args)
        self._jax.block_until_ready(outs)
        return outs

    def results(self, outs):
        return [
            {name: np.asarray(outs[i]).reshape(
                self.n_cores, *self.out_avals[i].shape)[c]
             for i, name in enumerate(self.out_names)}
            for c in range(self.n_cores)]


_RUNNER = None
_ARGS = None
TRACE = False

# ---------------------------------------------------------------------------
# Entry point
# ---------------------------------------------------------------------------

def kernel(emb, W1, a1, W2, a2, edges):
    global _RUNNER, _ARGS

    emb = np.asarray(emb)
    n_nodes = emb.shape[0]
    npc, stripe, _ = _cfg(n_nodes)

    xidx, scol, cap = _prep(np.asarray(edges), n_nodes)
    nc = _build_program(n_nodes, cap)

    in_maps = []
    for c in range(NCORES):
        embT = np.zeros((D, stripe), ml_dtypes.bfloat16)
        embT[:, :npc] = emb[c * npc:(c + 1) * npc].T.astype(ml_dtypes.bfloat16)
        in_maps.append({
            "embT": embT,
            "iotaf": _IOTA_F,
            "identbf": _IDENT_BF,
            "W1": np.asarray(W1).astype(ml_dtypes.bfloat16),
            "W2": np.asarray(W2).astype(ml_dtypes.bfloat16),
            "Wa1": np.stack([np.asarray(W1) @ np.asarray(a1)[:D],
                             np.asarray(W1) @ np.asarray(a1)[D:]],
                            1).astype(ml_dtypes.bfloat16),
            "Wa2": np.stack([np.asarray(W2) @ np.asarray(a2)[:D],
                             np.asarray(W2) @ np.asarray(a2)[D:]],
                            1).astype(ml_dtypes.bfloat16),
            "xidx": xidx[c], "scol": scol[c],
        })

    runner = _Runner(nc, NCORES)
    args = runner.prepare(in_maps)
    results = runner.results(runner.run(args))
    _RUNNER, _ARGS = runner, args
    out = np.concatenate(
        [results[c]["out_stripe"][:npc] for c in range(NCORES)], 0)
    return out.astype(np.float32)
